# revision 3
# baseline (speedup 1.0000x reference)
"""ChebConv(K=2) x2 GNN forward on 8 Trainium2 NeuronCores.

Strategy (sharding_hint: shard nodes + edges by dst, replicate weights):
  - Nodes row-sharded: core c owns rows [c*RPC, (c+1)*RPC), padded to RPC_PAD.
  - Edges partitioned by dst ownership, sorted by dst-block (128 rows), padded
    to 128-edge groups; per-(block) group counts equalized across cores so one
    SPMD program serves all 8 cores.
  - prop(h) = segment_sum(norm * h[src]) computed as, per dst-block:
      PSUM[c, dst] += sum_groups  G_g[e, c]^T-contracted one-hot:
      matmul(lhsT=G_g [128e x C], rhs=S_g [128e x 128dst]) where
      S_g[e, d] = (iota[d] == dst_local[e]) * norm[e]   (one DVE tensor_scalar)
      G_g gathered from HBM via indirect_dma_start (128 rows per call).
  - Layer 1 gathers rows of y1 = x @ W1[1] (each core computes full y1: the
    12.8MB bf16 all-gather it replaces is slower than recompute).
  - Layer 2 gathers rows of y2 = h @ W2[1] ([N,64]); y2 is AllGather'd
    (bf16, 6.4MB) since h only exists sharded.
  - Dense terms x@W1[0], h@W2[0] + biases accumulate into the same PSUM tiles.
  - All matmuls bf16 (f32 accumulate in PSUM).
"""
import sys
sys.path.insert(0, "/opt/trn_rl_repo")
import numpy as np

import concourse.bacc as bacc
import concourse.bass as bass
import concourse.mybir as mybir
import concourse.tile as tile
from concourse import bass_utils

NCORES = 8
P = 128
BF16 = mybir.dt.bfloat16
F32 = mybir.dt.float32
I32 = mybir.dt.int32
NPBF16 = mybir.dt.np(BF16)

_PROG_CACHE = {}


# ---------------------------------------------------------------- host prep
def _host_prep(x, edge_index, edge_weight):
    N, CIN = x.shape
    E = edge_index.shape[1]
    src = np.asarray(edge_index[0], dtype=np.int64)
    dst = np.asarray(edge_index[1], dtype=np.int64)
    w = np.asarray(edge_weight, dtype=np.float64)

    deg = np.bincount(src, weights=w, minlength=N).astype(np.float32)
    dinv = np.where(deg > 0, 1.0 / np.sqrt(np.where(deg > 0, deg, 1.0)), 0.0).astype(np.float32)
    norm = (-dinv[src] * edge_weight.astype(np.float32) * dinv[dst]).astype(np.float32)

    RPC = -(-N // NCORES)                 # rows per core (un-padded)
    NB = -(-RPC // P)                     # dst blocks per core
    RPC_PAD = NB * P
    NPAD = RPC_PAD * NCORES

    core_of = dst // RPC
    blk_of = (dst - core_of * RPC) // P

    # bucket edges by (core, block)
    order = np.lexsort((src, blk_of, core_of))
    src_s, dst_s, norm_s = src[order], dst[order], norm[order]
    core_s, blk_s = core_of[order], blk_of[order]

    # per (core, block) counts -> equalized group counts
    counts = np.zeros((NCORES, NB), dtype=np.int64)
    np.add.at(counts, (core_s, blk_s), 1)
    gcounts = -(-counts // P)             # groups needed per (core, block)
    gmax = gcounts.max(axis=0)            # equalized groups per block
    gmax = np.maximum(gmax, 1)
    GTOT = int(gmax.sum())                # total groups per core

    # packed arrays [NCORES][128, GTOT]
    idx1 = np.zeros((NCORES, P, GTOT), dtype=np.int32)
    idx2 = np.zeros((NCORES, P, GTOT), dtype=np.int32)
    dstl = np.zeros((NCORES, P, GTOT), dtype=np.float32)
    nrm = np.zeros((NCORES, P, GTOT), dtype=np.float32)

    # y2full row index of node n (per-core padded concat)
    n_core = np.arange(N) // RPC
    y2row = (n_core * RPC_PAD + (np.arange(N) - n_core * RPC)).astype(np.int32)

    # slice boundaries of each (core, block) run inside the sorted arrays
    starts = np.zeros((NCORES, NB + 1), dtype=np.int64)
    flat = core_s * NB + blk_s
    bnd = np.searchsorted(flat, np.arange(NCORES * NB + 1))
    goff = np.concatenate([[0], np.cumsum(gmax)]).astype(np.int64)

    for c in range(NCORES):
        for b in range(NB):
            lo, hi = bnd[c * NB + b], bnd[c * NB + b + 1]
            ne = hi - lo
            g0 = goff[b]
            if ne == 0:
                continue
            sl = slice(lo, hi)
            e_src = src_s[sl]
            e_dstl = (dst_s[sl] - c * RPC - b * P).astype(np.float32)
            e_nrm = norm_s[sl]
            # positions j = 0..ne-1 -> (partition j%128, group g0 + j//128)
            jj = np.arange(ne)
            pp = jj % P
            gg = g0 + jj // P
            idx1[c, pp, gg] = e_src
            idx2[c, pp, gg] = y2row[e_src]
            dstl[c, pp, gg] = e_dstl
            nrm[c, pp, gg] = e_nrm

    meta = dict(N=N, E=E, CIN=CIN, RPC=RPC, NB=NB, RPC_PAD=RPC_PAD, NPAD=NPAD,
                gmax=tuple(int(g) for g in gmax), GTOT=GTOT)
    packs = dict(idx1=idx1, idx2=idx2, dstl=dstl, nrm=nrm)
    return meta, packs


# ---------------------------------------------------------------- program
def _build_program(meta, HID, COUT):
    N, CIN = meta["N"], meta["CIN"]
    NB, RPC_PAD, NPAD = meta["NB"], meta["RPC_PAD"], meta["NPAD"]
    gmax, GTOT = meta["gmax"], meta["GTOT"]
    CH_T = 16                      # y1 tiles per xT chunk
    CHCOLS = CH_T * P

    nc = bacc.Bacc("TRN2", target_bir_lowering=False, debug=False,
                   num_devices=NCORES)
    xT = nc.dram_tensor("xT", [P, NPAD], BF16, kind="ExternalInput")
    xlocT = nc.dram_tensor("xlocT", [P, RPC_PAD], BF16, kind="ExternalInput")
    W1_0 = nc.dram_tensor("W1_0", [CIN, HID], BF16, kind="ExternalInput")
    W1_1 = nc.dram_tensor("W1_1", [CIN, HID], BF16, kind="ExternalInput")
    W2_0 = nc.dram_tensor("W2_0", [HID, COUT], BF16, kind="ExternalInput")
    W2_1 = nc.dram_tensor("W2_1", [HID, COUT], BF16, kind="ExternalInput")
    b1t = nc.dram_tensor("b1", [HID, 1], F32, kind="ExternalInput")
    b2t = nc.dram_tensor("b2", [COUT, 1], F32, kind="ExternalInput")
    idx1t = nc.dram_tensor("idx1", [P, GTOT], I32, kind="ExternalInput")
    idx2t = nc.dram_tensor("idx2", [P, GTOT], I32, kind="ExternalInput")
    dstlt = nc.dram_tensor("dstl", [P, GTOT], F32, kind="ExternalInput")
    nrmt = nc.dram_tensor("nrm", [P, GTOT], F32, kind="ExternalInput")
    outT = nc.dram_tensor("outT", [COUT, RPC_PAD], F32, kind="ExternalOutput")

    y1hbm = nc.dram_tensor("y1hbm", [NPAD, HID], BF16, kind="Internal")
    y2agin = nc.dram_tensor("y2agin", [RPC_PAD, COUT], BF16, kind="Internal")
    y2full = nc.dram_tensor("y2full", [NPAD, COUT], BF16, kind="Internal",
                            addr_space="Shared")

    with tile.TileContext(nc) as tc:
        with (
            tc.tile_pool(name="const", bufs=1) as cpool,
            tc.tile_pool(name="xchunk", bufs=3) as xpool,
            tc.tile_pool(name="y1st", bufs=3) as ypool,
            tc.tile_pool(name="gat", bufs=6) as gpool,
            tc.tile_pool(name="stp", bufs=6) as spool,
            tc.tile_pool(name="hT", bufs=1) as hpool,
            tc.tile_pool(name="oS", bufs=3) as opool,
            tc.tile_pool(name="ps", bufs=2, space="PSUM") as pspool,
        ):
            # ---- constants
            iota = cpool.tile([P, P], F32)
            nc.gpsimd.iota(iota[:], pattern=[[1, P]], base=0,
                           channel_multiplier=0,
                           allow_small_or_imprecise_dtypes=True)
            w10 = cpool.tile([CIN, HID], BF16)
            w11 = cpool.tile([CIN, HID], BF16)
            w20 = cpool.tile([HID, COUT], BF16)
            w21 = cpool.tile([HID, COUT], BF16)
            b1s = cpool.tile([HID, 1], F32)
            b2s = cpool.tile([COUT, 1], F32)
            for t, d in ((w10, W1_0), (w11, W1_1), (w20, W2_0), (w21, W2_1),
                         (b1s, b1t), (b2s, b2t)):
                nc.sync.dma_start(t[:], d[:])
            idx1s = cpool.tile([P, GTOT], I32)
            idx2s = cpool.tile([P, GTOT], I32)
            dstls = cpool.tile([P, GTOT], F32)
            nrms = cpool.tile([P, GTOT], F32)
            nc.sync.dma_start(idx1s[:], idx1t[:])
            nc.sync.dma_start(idx2s[:], idx2t[:])
            nc.sync.dma_start(dstls[:], dstlt[:])
            nc.sync.dma_start(nrms[:], nrmt[:])
            xloc = cpool.tile([P, RPC_PAD], BF16)
            nc.sync.dma_start(xloc[:], xlocT[:])
            hT = hpool.tile([HID, RPC_PAD], BF16)

            # ---- phase A: y1 = x @ W1[1] (full, replicated)
            nchunk = -(-NPAD // CHCOLS)
            for ch in range(nchunk):
                c0 = ch * CHCOLS
                cols = min(CHCOLS, NPAD - c0)
                nt = cols // P
                xc = xpool.tile([P, CHCOLS], BF16, tag="xc")
                nc.sync.dma_start(xc[:, :cols], xT[:, c0:c0 + cols])
                yst = ypool.tile([P, CH_T, HID], BF16, tag="yst")
                for t in range(nt):
                    ps = pspool.tile([P, HID], F32, tag="psA")
                    nc.tensor.matmul(ps[:], xc[:, t * P:(t + 1) * P], w11[:],
                                     start=True, stop=True)
                    eng = nc.scalar if (t % 2 == 0) else nc.vector
                    if eng is nc.scalar:
                        eng.copy(yst[:, t, :], ps[:])
                    else:
                        eng.tensor_copy(yst[:, t, :], ps[:])
                # rows r = t*128 + p  ->  HBM AP [p, t, c]
                dst_ap = y1hbm.ap()[c0:c0 + cols, :].rearrange(
                    "(t p) c -> p t c", p=P)
                nc.sync.dma_start(dst_ap, yst[:, :nt, :])

            # ---- phase B: h^T = relu(W1[0]^T xlocT + prop1 + b1)
            g_base = 0
            for b in range(NB):
                ng = gmax[b]
                psB = pspool.tile([HID, P], F32, tag="psB")
                nc.tensor.matmul(psB[:], w10[:], xloc[:, b * P:(b + 1) * P],
                                 start=True, stop=(ng == 0))
                for g in range(ng):
                    j = g_base + g
                    gt = gpool.tile([P, HID], BF16, tag="g1")
                    nc.gpsimd.indirect_dma_start(
                        out=gt[:], out_offset=None, in_=y1hbm.ap(),
                        in_offset=bass.IndirectOffsetOnAxis(
                            ap=idx1s[:, j:j + 1], axis=0))
                    st = spool.tile([P, P], BF16, tag="s1")
                    nc.vector.tensor_scalar(
                        st[:], iota[:], dstls[:, j:j + 1], nrms[:, j:j + 1],
                        op0=mybir.AluOpType.is_equal, op1=mybir.AluOpType.mult)
                    nc.tensor.matmul(psB[:], gt[:], st[:],
                                     start=False, stop=(g == ng - 1))
                g_base += ng
                nc.scalar.activation(hT[:, b * P:(b + 1) * P], psB[:],
                                     mybir.ActivationFunctionType.Relu,
                                     bias=b1s[:], scale=1.0)

            # ---- phase C: y2 = h @ W2[1] -> AllGather
            for b in range(NB):
                psC = pspool.tile([P, COUT], F32, tag="psC")
                nc.tensor.matmul(psC[:], hT[:, b * P:(b + 1) * P], w21[:],
                                 start=True, stop=True)
                y2s = opool.tile([P, COUT], BF16, tag="y2s")
                eng = nc.scalar if (b % 2 == 0) else nc.vector
                if eng is nc.scalar:
                    eng.copy(y2s[:], psC[:])
                else:
                    eng.tensor_copy(y2s[:], psC[:])
                nc.sync.dma_start(y2agin.ap()[b * P:(b + 1) * P, :], y2s[:])
            nc.gpsimd.collective_compute(
                "AllGather", mybir.AluOpType.bypass,
                replica_groups=[list(range(NCORES))],
                ins=[y2agin.ap()], outs=[y2full.ap()])

            # ---- phase D: out^T = W2[0]^T hT + prop2 + b2
            g_base = 0
            for b in range(NB):
                ng = gmax[b]
                psD = pspool.tile([COUT, P], F32, tag="psD")
                nc.tensor.matmul(psD[:], w20[:], hT[:, b * P:(b + 1) * P],
                                 start=True, stop=(ng == 0))
                for g in range(ng):
                    j = g_base + g
                    gt2 = gpool.tile([P, COUT], BF16, tag="g2")
                    nc.gpsimd.indirect_dma_start(
                        out=gt2[:], out_offset=None, in_=y2full.ap(),
                        in_offset=bass.IndirectOffsetOnAxis(
                            ap=idx2s[:, j:j + 1], axis=0))
                    st2 = spool.tile([P, P], BF16, tag="s2")
                    nc.vector.tensor_scalar(
                        st2[:], iota[:], dstls[:, j:j + 1], nrms[:, j:j + 1],
                        op0=mybir.AluOpType.is_equal, op1=mybir.AluOpType.mult)
                    nc.tensor.matmul(psD[:], gt2[:], st2[:],
                                     start=False, stop=(g == ng - 1))
                g_base += ng
                oT = opool.tile([COUT, P], F32, tag="oT")
                nc.scalar.activation(oT[:], psD[:],
                                     mybir.ActivationFunctionType.Identity,
                                     bias=b2s[:], scale=1.0)
                nc.sync.dma_start(outT.ap()[:, b * P:(b + 1) * P], oT[:])
    nc.compile()
    return nc


# ---------------------------------------------------------------- kernel
def kernel(x, edge_index, edge_weight, W1, b1, W2, b2):
    x = np.asarray(x)
    N, CIN = x.shape
    K, _, HID = np.asarray(W1).shape
    COUT = np.asarray(W2).shape[2]
    assert K == 2

    meta, packs = _host_prep(x, np.asarray(edge_index), np.asarray(edge_weight))
    RPC, RPC_PAD, NPAD = meta["RPC"], meta["RPC_PAD"], meta["NPAD"]

    key = (N, CIN, HID, COUT, meta["gmax"])
    if key not in _PROG_CACHE:
        _PROG_CACHE[key] = _build_program(meta, HID, COUT)
    nc = _PROG_CACHE[key]

    # full padded x^T (pad at the end; y1 row n == node n for n < N)
    xTfull = np.zeros((CIN, NPAD), dtype=NPBF16)
    xTfull[:, :N] = x.astype(NPBF16).T
    W1b = np.asarray(W1).astype(NPBF16)
    W2b = np.asarray(W2).astype(NPBF16)
    b1c = np.asarray(b1, dtype=np.float32).reshape(HID, 1)
    b2c = np.asarray(b2, dtype=np.float32).reshape(COUT, 1)

    in_maps = []
    for c in range(NCORES):
        lo = c * RPC
        xloc = np.zeros((CIN, RPC_PAD), dtype=NPBF16)
        hi = min(N, lo + RPC)
        if hi > lo:
            xloc[:, :hi - lo] = x[lo:hi].astype(NPBF16).T
        in_maps.append({
            "xT": xTfull, "xlocT": xloc,
            "W1_0": W1b[0], "W1_1": W1b[1],
            "W2_0": W2b[0], "W2_1": W2b[1],
            "b1": b1c, "b2": b2c,
            "idx1": packs["idx1"][c], "idx2": packs["idx2"][c],
            "dstl": packs["dstl"][c],
            "nrm": packs["nrm"][c],
        })

    res = bass_utils.run_bass_kernel_spmd(nc, in_maps, core_ids=list(range(NCORES)))

    out = np.empty((N, COUT), dtype=np.float32)
    for c in range(NCORES):
        lo = c * RPC
        hi = min(N, lo + RPC)
        if hi > lo:
            out[lo:hi] = res.results[c]["outT"][:, :hi - lo].T
    return out
